# revision 2
# baseline (speedup 1.0000x reference)
"""Trainium2 kernel for: out = tanh(x @ scatter_nd(nonzero_ind, kernel_vector, (20000, 4096)) + bias).

Strategy v2 (8 NeuronCores, unit-sharded):
  - Host builds the dense (20096, 4096) fp16 weight matrix (K padded 20000->20096
    with zero rows) and the fp16 transposed activations xt (20096, 2048).
  - Core c owns the 512-unit column slice W[:, c*512:(c+1)*512] (20.6MB), kept
    SBUF-resident as 157 [128 x 512] fp16 tiles, loaded interleaved with the
    first batch-block's k-loop.
  - xt streams through as [128 x 512] fp16 moving tiles; per (batch-block bb,
    k-tile kt): 4 matmuls psum[bt] += xt_tile[:, bt*128:(bt+1)*128].T @ w[kt],
    accumulating over all 157 k-tiles in 4 PSUM banks (double-buffered across
    batch-blocks).
  - 4 bb x 157 kt x 4 bt = 2512 matmuls of [128x128]x[128x512] per core
    (the k x4 / batch x2 sharding needs 2560 due to k-padding to 160 tiles).
  - Host applies bias + tanh; output slices concatenate directly (no partial
    reduction).
"""

import numpy as np

P = 128
B, K, U = 2048, 20000, 4096
KPAD = 20096             # 157 * 128
KT = KPAD // P           # 157 k-tiles
USH = U // 8             # 512 units per core
BBLK = 512               # batch block (moving-tile width)
NBB = B // BBLK          # 4 batch blocks
NBT = BBLK // P          # 4 batch sub-tiles -> 4 PSUM banks live
WPREF = 6                # W-tile DMA prefetch depth during bb=0

TRACE = False            # set by test harness for profiled runs
LAST_RESULT = None       # BassKernelResults of the last run (for the harness)

_NC_CACHE = {}


def _build_nc():
    from concourse import bacc
    import concourse.mybir as mybir
    import concourse.tile as tile

    f32 = mybir.dt.float32
    f16 = mybir.dt.float16

    nc = bacc.Bacc("TRN2", target_bir_lowering=False, debug=False)
    xt_d = nc.dram_tensor("xt", [KPAD, B], f16, kind="ExternalInput").ap()
    w_d = nc.dram_tensor("w_sh", [KPAD, USH], f16, kind="ExternalInput").ap()
    o_d = nc.dram_tensor("out_p", [B, USH], f32, kind="ExternalOutput").ap()

    with tile.TileContext(nc) as tc:
        with (
            tc.tile_pool(name="wres", bufs=1) as wpool,
            tc.tile_pool(name="xstream", bufs=6) as xpool,
            tc.tile_pool(name="stage", bufs=4) as spool,
            tc.tile_pool(name="mpsum", bufs=2, space="PSUM") as mpsum,
        ):
            # Resident W tiles; DMAs are interleaved with bb=0's k-loop
            # (prefetched WPREF tiles ahead) so the PE starts immediately.
            w = [
                wpool.tile([P, USH], f16, tag=f"w{kt}", name=f"w{kt}")
                for kt in range(KT)
            ]

            for bb in range(NBB):
                psums = [
                    mpsum.tile([P, BBLK], f32, tag=f"ps{bt}", name=f"ps{bt}")
                    for bt in range(NBT)
                ]
                for kt in range(KT):
                    if bb == 0:
                        if kt == 0:
                            for j in range(WPREF):
                                nc.sync.dma_start(
                                    w[j][:], w_d[j * P:(j + 1) * P, :]
                                )
                        if kt + WPREF < KT:
                            j = kt + WPREF
                            nc.sync.dma_start(
                                w[j][:], w_d[j * P:(j + 1) * P, :]
                            )
                    xt = xpool.tile([P, BBLK], f16, tag="xt", name="xt")
                    nc.sync.dma_start(
                        xt[:],
                        xt_d[kt * P:(kt + 1) * P, bb * BBLK:(bb + 1) * BBLK],
                    )
                    for bt in range(NBT):
                        nc.tensor.matmul(
                            psums[bt][:],
                            xt[:, bt * P:(bt + 1) * P],
                            w[kt][:],
                            start=(kt == 0),
                            stop=(kt == KT - 1),
                        )
                for bt in range(NBT):
                    st = spool.tile([P, BBLK], f32, tag="st", name="st")
                    nc.vector.tensor_copy(st[:], psums[bt][:])
                    nc.sync.dma_start(
                        o_d[bb * BBLK + bt * P:bb * BBLK + (bt + 1) * P, :],
                        st[:],
                    )

    nc.compile()
    return nc


def _get_nc():
    if "v2" not in _NC_CACHE:
        _NC_CACHE["v2"] = _build_nc()
    return _NC_CACHE["v2"]


def kernel(x, kernel_vector, bias, nonzero_ind):
    global LAST_RESULT
    from concourse.bass_utils import run_bass_kernel_spmd

    x = np.asarray(x, dtype=np.float32)
    kernel_vector = np.asarray(kernel_vector, dtype=np.float32)
    bias = np.asarray(bias, dtype=np.float32)
    nonzero_ind = np.asarray(nonzero_ind)

    nc = _get_nc()

    # Host scatter: dense fp16 weights, K padded with zero rows to 157 tiles.
    rows = nonzero_ind[:, 0].astype(np.int64)
    cols = nonzero_ind[:, 1].astype(np.int64)
    w_acc = np.zeros(K * U, np.float32)
    np.add.at(w_acc, rows * U + cols, kernel_vector)
    w_full = np.zeros((KPAD, U), np.float16)
    w_full[:K] = w_acc.reshape(K, U)

    # Transposed, padded activations shared by all cores.
    xt_full = np.zeros((KPAD, B), np.float16)
    xt_full[:K] = x.T

    in_maps = [
        {"xt": xt_full, "w_sh": np.ascontiguousarray(w_full[:, c * USH:(c + 1) * USH])}
        for c in range(8)
    ]

    kwargs = {}
    if TRACE:
        kwargs = dict(trace=True, trace_cores=list(range(8)))
    res = run_bass_kernel_spmd(nc, in_maps, core_ids=list(range(8)), **kwargs)
    LAST_RESULT = res

    out = np.empty((B, U), np.float32)
    for c in range(8):
        out[:, c * USH:(c + 1) * USH] = res.results[c]["out_p"]
    out += bias[None, :]
    np.tanh(out, out=out)
    return out


# revision 3
# speedup vs baseline: 1.1933x; 1.1933x over previous
"""Trainium2 kernel for: out = tanh(x @ scatter_nd(nonzero_ind, kernel_vector, (20000, 4096)) + bias).

Strategy v3 (8 NeuronCores, unit-sharded, chunked k-stream):
  - Core c owns the 512-unit column slice W[:, c*512:(c+1)*512]. Both operands
    stream from HBM exactly once per core: x as transposed fp16 [128 x 2048]
    tiles (contiguous 512KB DMAs on the sync HWDGE ring), W as fp16 [128 x 512]
    tiles (contiguous 128KB DMAs on the scalar HWDGE ring). Big DMAs matter:
    each dma_start costs ~600ns of engine issue time, so per-(kt,bb) 128KB x
    loads (628 of them) serialize into ~480us of issue and starve the PE.
  - k-tiles are processed in chunks (sizes [5, 8, 12 x 12] = 157). Within a
    chunk: for each of 4 batch-blocks, 4 matmuls per k-tile accumulate in 4
    PSUM banks (double-buffered); the chunk partial is then folded into 16
    SBUF fp32 accumulators by the vector engine (copy on first chunk, add
    after, add-into-stage + DMA out on the last chunk).
  - 157 kt x 4 bb x 4 bt = 2512 matmuls of [128x128]x[128x512] per core at
    the fp16 PE roofline (~216.8 ns each).
  - Host applies bias + tanh; output slices concatenate (no reduction).
"""

import numpy as np

P = 128
B, K, U = 2048, 20000, 4096
KPAD = 20096             # 157 * 128
KT = KPAD // P           # 157 k-tiles
USH = U // 8             # 512 units per core
BBLK = 512               # batch block
NBB = B // BBLK          # 4 batch blocks
NBT = BBLK // P          # 4 batch sub-tiles -> 4 live PSUM banks
CHUNKS = [5, 8] + [12] * 12          # sums to 157; small first chunks hide the
MAXC = max(CHUNKS)                   # x-DMA ramp while chunk 1+ prefetches

TRACE = False            # set by test harness for profiled runs
LAST_RESULT = None       # BassKernelResults of the last run (for the harness)

_NC_CACHE = {}


def _build_nc():
    from concourse import bacc
    import concourse.mybir as mybir
    import concourse.tile as tile

    f32 = mybir.dt.float32
    f16 = mybir.dt.float16

    nc = bacc.Bacc("TRN2", target_bir_lowering=False, debug=False)
    xt_d = nc.dram_tensor("xt", [KPAD, B], f16, kind="ExternalInput").ap()
    w_d = nc.dram_tensor("w_sh", [KPAD, USH], f16, kind="ExternalInput").ap()
    o_d = nc.dram_tensor("out_p", [B, USH], f32, kind="ExternalOutput").ap()

    with tile.TileContext(nc) as tc:
        with (
            tc.tile_pool(name="xstream", bufs=2) as xpool,
            tc.tile_pool(name="wstream", bufs=2) as wpool,
            tc.tile_pool(name="accum", bufs=1) as apool,
            tc.tile_pool(name="stage", bufs=4) as spool,
            tc.tile_pool(name="mpsum", bufs=2, space="PSUM") as mpsum,
        ):
            acc = [
                apool.tile([P, USH], f32, tag=f"acc{i}", name=f"acc{i}")
                for i in range(NBB * NBT)
            ]

            nchunks = len(CHUNKS)
            k0 = 0
            for c, kc in enumerate(CHUNKS):
                xc, wc = [], []
                for j in range(kc):
                    kt = k0 + j
                    xt = xpool.tile([P, B], f16, tag=f"x{j}", name=f"x{j}")
                    nc.sync.dma_start(
                        xt[:], xt_d[kt * P:(kt + 1) * P, :]
                    )
                    wt = wpool.tile([P, USH], f16, tag=f"w{j}", name=f"w{j}")
                    nc.scalar.dma_start(
                        wt[:], w_d[kt * P:(kt + 1) * P, :]
                    )
                    xc.append(xt)
                    wc.append(wt)

                for bb in range(NBB):
                    psums = [
                        mpsum.tile([P, BBLK], f32, tag=f"ps{bt}", name=f"ps{bt}")
                        for bt in range(NBT)
                    ]
                    for j in range(kc):
                        for bt in range(NBT):
                            nc.tensor.matmul(
                                psums[bt][:],
                                xc[j][:, bb * BBLK + bt * P:bb * BBLK + (bt + 1) * P],
                                wc[j][:],
                                start=(j == 0),
                                stop=(j == kc - 1),
                            )
                    for bt in range(NBT):
                        a = acc[bb * NBT + bt]
                        if c == 0:
                            nc.vector.tensor_copy(a[:], psums[bt][:])
                        elif c < nchunks - 1:
                            nc.vector.tensor_add(a[:], psums[bt][:], a[:])
                        else:
                            st = spool.tile([P, BBLK], f32, tag="st", name="st")
                            nc.vector.tensor_add(st[:], psums[bt][:], a[:])
                            nc.sync.dma_start(
                                o_d[bb * BBLK + bt * P:bb * BBLK + (bt + 1) * P, :],
                                st[:],
                            )
                k0 += kc

    nc.compile()
    return nc


def _get_nc():
    if "v3" not in _NC_CACHE:
        _NC_CACHE["v3"] = _build_nc()
    return _NC_CACHE["v3"]


def kernel(x, kernel_vector, bias, nonzero_ind):
    global LAST_RESULT
    from concourse.bass_utils import run_bass_kernel_spmd

    x = np.asarray(x, dtype=np.float32)
    kernel_vector = np.asarray(kernel_vector, dtype=np.float32)
    bias = np.asarray(bias, dtype=np.float32)
    nonzero_ind = np.asarray(nonzero_ind)

    nc = _get_nc()

    # Host scatter: dense fp16 weights, K padded with zero rows to 157 tiles.
    rows = nonzero_ind[:, 0].astype(np.int64)
    cols = nonzero_ind[:, 1].astype(np.int64)
    w_acc = np.zeros(K * U, np.float32)
    np.add.at(w_acc, rows * U + cols, kernel_vector)
    w_full = np.zeros((KPAD, U), np.float16)
    w_full[:K] = w_acc.reshape(K, U)

    # Transposed, padded activations shared by all cores.
    xt_full = np.zeros((KPAD, B), np.float16)
    xt_full[:K] = x.T

    in_maps = [
        {"xt": xt_full, "w_sh": np.ascontiguousarray(w_full[:, c * USH:(c + 1) * USH])}
        for c in range(8)
    ]

    kwargs = {}
    if TRACE:
        kwargs = dict(trace=True, trace_cores=list(range(8)))
    res = run_bass_kernel_spmd(nc, in_maps, core_ids=list(range(8)), **kwargs)
    LAST_RESULT = res

    out = np.empty((B, U), np.float32)
    for c in range(8):
        out[:, c * USH:(c + 1) * USH] = res.results[c]["out_p"]
    out += bias[None, :]
    np.tanh(out, out=out)
    return out
